# revision 7
# baseline (speedup 1.0000x reference)
"""TRN2 Bass kernel for gnn_message_passing (nn_Model_34823594836411).

Math (matches reference.py):
  per edge e: rel = pos[dst] - pos[src]; sh1 = rel / max(|rel|, 1e-12)
  out[n, 0]   = w0 * f[n] * c_n / max(c_n, 1)
  out[n, 1:4] = w1 * f[n] * segsum(sh1)_n / max(c_n, 1)
where f = node_feat[:, 0] and c_n = in-degree of node n (s = node_feat[dst]
is constant within a segment, so it factors out of the edge sums).

Strategy: dst-shard nodes across 8 cores (12544/core). Each node owns a
padded row of C slots (C = pow2 >= max degree); padding slots use src=dst
so rel=0 contributes nothing. The only random access is the src-position
gather, executed with the ANT dma_gather SWDGE ucode: positions are packed
4 nodes per 256B DRAM record (48B payload), so idx = src>>2 <= 25088 fits
int16 in a single window; the right 12B sub-record is selected on-chip
with two host-shipped 0/1 masks. p_dst needs no gather (per-node broadcast
along the C slots via a step-0 AP). Segment-sum = log2(C) halving adds.
All float arithmetic happens on device; the host only sorts/packs indices
and re-lays-out input tensors.
"""
import numpy as np

import concourse.bacc as bacc
import concourse.bass as bass
import concourse.mybir as mybir
from concourse import library_config
from concourse.bass_utils import run_bass_kernel_spmd
from concourse._compat import exact_div

N_NODES = 100000
N_EDGES = 3200000
NC = 8
P = 128
NPC = 12544            # nodes per core (98 blocks of 128); 8*12544 = 100352
B = NPC // P           # 98 blocks
NREC = (NC * NPC) // 4  # 25088 4-node records in the position table
EPS2 = 1e-24
CALL_IDX = 1024        # gather idxs per dma_gather call (ring-capacity safe)


def set_mini(n_nodes, nc_, npc):
    """Shrink the problem for CoreSim debugging."""
    global N_NODES, NC, NPC, B, NREC
    N_NODES, NC, NPC = n_nodes, nc_, npc
    B = NPC // P
    NREC = (NC * NPC) // 4

F32 = mybir.dt.float32
I16 = mybir.dt.int16


def _ap(t, off, dims):
    return bass.AP(t, off, dims)


def dma_gather_raw(gpsimd, out_ap, in_ap, idxs_ap, num_idxs, elem_size,
                   elem_step):
    """Non-transpose DRAM-source InstDMAGatherAnt without the 256B-elem
    assert: out[i % 128, i // 128, :] = table[idx[i], :elem_size]."""
    stride_bytes_256 = exact_div(elem_step * 4, 256)
    return gpsimd.add_instruction(
        mybir.InstDMAGatherAnt(
            name=gpsimd.bass.get_next_instruction_name(),
            ins=[
                *gpsimd.lower_ap_dma(in_ap, for_custom_bir_dma=True),
                gpsimd.lower_ap(idxs_ap),
                gpsimd.lower_val_access(gpsimd.to_reg(num_idxs)),
            ],
            outs=[gpsimd.lower_ap(out_ap)],
            transpose=False,
            num_idxs=num_idxs,
            elem_size=elem_size,
            stride_bytes_256=stride_bytes_256,
            gen_mode=0,
            single_packet=True,
            queue_num=0,
            sbuf_tokens_per_rank=0,
            sbuf_free_dim_per_rank=0,
            sbuf_free_dim_pad_per_rank=0,
            sbuf_byte_offset=0,
        )
    )


_PROG_CACHE = {}


def build_program(C, chunk_blocks):
    key = (C, chunk_blocks)
    if key in _PROG_CACHE:
        return _PROG_CACHE[key]

    AL = mybir.AluOpType
    cols = B * C
    n_chunks = B // chunk_blocks
    assert n_chunks * chunk_blocks == B
    ch_cols = chunk_blocks * C
    ch_idx = ch_cols * P
    calls = ch_idx // CALL_IDX
    assert calls * CALL_IDX == ch_idx
    ccols = CALL_IDX // P             # record columns written per call

    nc = bacc.Bacc("TRN2")
    # register the sqrt-bias constant (mimics Bass.__init__ const AP setup)
    _eps_t = nc.alloc_sbuf_tensor("const-float32-eps2", [128, 1], F32)
    nc.gpsimd.memset(_eps_t.ap(), EPS2)
    nc.const_aps.aps[(F32, EPS2)] = _eps_t.ap()
    nc.all_engine_barrier()

    ptab = nc.dram_tensor("ptab", [NREC, 64], F32, kind="ExternalInput")
    idxs = nc.dram_tensor("idxs", [128, cols * P // 16], I16, kind="ExternalInput")
    masks = nc.dram_tensor("masks", [128, 4, cols], F32, kind="ExternalInput")
    pdst = nc.dram_tensor("pdst", [128, B, 3], F32, kind="ExternalInput")
    cnts = nc.dram_tensor("cnts", [128, B], F32, kind="ExternalInput")
    nfeat = nc.dram_tensor("nfeat", [128, B], F32, kind="ExternalInput")
    wvec = nc.dram_tensor("wvec", [128, 4], F32, kind="ExternalInput")
    out = nc.dram_tensor("out", [128, B, 4], F32, kind="ExternalOutput")

    tab_ap = _ap(ptab, 0, [[64, NREC], [1, 12]])

    # semaphore schedule (all counts computed identically on every engine):
    # g_sem: +16 per DMA/gather issued by gpsimd
    # a_sem: +1 by vector when chunk's ss ready (value 2ch+1),
    #        +1 by scalar when chunk's inv ready (value 2ch+2)
    # v_sem: +1 by vector when chunk fully consumed (value ch+1),
    #        +1 more after the final combine
    g_after_static = 4 * 16
    g_per_chunk = 2 * 16 + calls * 16

    def g_after(ch):
        return g_after_static + (ch + 1) * g_per_chunk

    with (
        nc.sbuf_tensor("idx_sb", [128, ch_idx // 16], I16) as idx_sb,
        nc.sbuf_tensor("rec_sb", [128, ch_cols, 12], F32) as rec_sb,
        nc.sbuf_tensor("mk_sb", [128, 4, ch_cols], F32) as mk_sb,
        nc.sbuf_tensor("pa_sb", [128, ch_cols, 3], F32) as pa_sb,
        nc.sbuf_tensor("pb_sb", [128, ch_cols, 3], F32) as pb_sb,
        nc.sbuf_tensor("ss_sb", [128, ch_cols], F32) as ss_sb,
        nc.sbuf_tensor("inv_sb", [128, ch_cols], F32) as inv_sb,
        nc.sbuf_tensor("pdst_sb", [128, B, 3], F32) as pdst_sb,
        nc.sbuf_tensor("sums_sb", [128, B, 3], F32) as sums_sb,
        nc.sbuf_tensor("cnt_sb", [128, B], F32) as cnt_sb,
        nc.sbuf_tensor("nf_sb", [128, B], F32) as nf_sb,
        nc.sbuf_tensor("w_sb", [128, 4], F32) as w_sb,
        nc.sbuf_tensor("o_sb", [128, B, 4], F32) as o_sb,
        nc.sbuf_tensor("t0_sb", [128, B], F32) as t0_sb,
        nc.sbuf_tensor("t1_sb", [128, B], F32) as t1_sb,
        nc.semaphore("g_sem") as g_sem,
        nc.semaphore("v_sem") as v_sem,
        nc.semaphore("a_sem") as a_sem,
        nc.Block() as block,
    ):
        @block.gpsimd
        def _(gpsimd):
            gpsimd.load_library(library_config.mlp)
            gpsimd.dma_start(pdst_sb[:], pdst[:]).then_inc(g_sem, 16)
            gpsimd.dma_start(cnt_sb[:], cnts[:]).then_inc(g_sem, 16)
            gpsimd.dma_start(nf_sb[:], nfeat[:]).then_inc(g_sem, 16)
            gpsimd.dma_start(w_sb[:], wvec[:]).then_inc(g_sem, 16)
            for ch in range(n_chunks):
                if ch >= 1:
                    # chunk buffers are single-buffered: wait for compute
                    gpsimd.wait_ge(v_sem, ch)
                iw = ch_idx // 16
                gpsimd.dma_start(
                    idx_sb[:], idxs[:, ch * iw:(ch + 1) * iw]
                ).then_inc(g_sem, 16)
                gpsimd.dma_start(
                    mk_sb[:], masks[:, :, ch * ch_cols:(ch + 1) * ch_cols]
                ).then_inc(g_sem, 16)
                gpsimd.wait_ge(g_sem, g_after(ch) - calls * 16)
                for k in range(calls):
                    dma_gather_raw(
                        gpsimd,
                        rec_sb[:, k * ccols:(k + 1) * ccols, :],
                        tab_ap,
                        idx_sb[:, k * (CALL_IDX // 16):(k + 1) * (CALL_IDX // 16)],
                        num_idxs=CALL_IDX, elem_size=12, elem_step=64,
                    ).then_inc(g_sem, 16)
            gpsimd.wait_ge(v_sem, n_chunks + 1)
            gpsimd.dma_start(out[:], o_sb[:]).then_inc(g_sem, 16)
            gpsimd.wait_ge(g_sem, g_after(n_chunks - 1) + 16)

        @block.vector
        def _(vector):
            for ch in range(n_chunks):
                vector.wait_ge(g_sem, g_after(ch))
                # exact select: psrc = sum_k rec_k * mask_k (three terms are
                # exact zeros, so the sum is bit-exact)
                def mk(kk):
                    return _ap(mk_sb, kk * ch_cols,
                               [[4 * ch_cols, 128], [1, ch_cols], [0, 3]])
                vector.tensor_tensor(out=pa_sb[:], in0=rec_sb[:, :, 0:3],
                                     in1=mk(0), op=AL.mult)
                for kk in range(1, 4):
                    vector.tensor_tensor(out=pb_sb[:],
                                         in0=rec_sb[:, :, 3 * kk:3 * kk + 3],
                                         in1=mk(kk), op=AL.mult)
                    vector.drain()
                    vector.tensor_tensor(out=pa_sb[:], in0=pa_sb[:], in1=pb_sb[:],
                                         op=AL.add)
                    vector.drain()
                # rel = pdst - psrc (in place, 4D APs)
                pd = _ap(pdst_sb, ch * chunk_blocks * 3,
                         [[B * 3, 128], [3, chunk_blocks], [0, C], [1, 3]])
                pa4 = _ap(pa_sb, 0,
                          [[ch_cols * 3, 128], [C * 3, chunk_blocks], [3, C], [1, 3]])
                vector.tensor_tensor(out=pa4, in0=pd, in1=pa4, op=AL.subtract)
                vector.drain()
                # ss = sum of squares over components
                vector.tensor_tensor(out=pb_sb[:], in0=pa_sb[:], in1=pa_sb[:],
                                     op=AL.mult)
                vector.drain()
                sq_x = _ap(pb_sb, 0, [[ch_cols * 3, 128], [3, ch_cols]])
                sq_y = _ap(pb_sb, 1, [[ch_cols * 3, 128], [3, ch_cols]])
                sq_z = _ap(pb_sb, 2, [[ch_cols * 3, 128], [3, ch_cols]])
                vector.tensor_tensor(out=ss_sb[:], in0=sq_x, in1=sq_y, op=AL.add)
                vector.drain()
                vector.tensor_tensor(out=ss_sb[:], in0=ss_sb[:], in1=sq_z,
                                     op=AL.add)
                vector.drain().then_inc(a_sem, 1)
                # sh = rel * rsqrt(ss + eps^2) once ACT publishes inv
                vector.wait_ge(a_sem, 2 * ch + 2)
                vector.reciprocal(out=inv_sb[:], in_=inv_sb[:])
                vector.drain()
                invb = _ap(inv_sb, 0, [[ch_cols, 128], [1, ch_cols], [0, 3]])
                vector.tensor_tensor(out=pa_sb[:], in0=pa_sb[:], in1=invb,
                                     op=AL.mult)
                vector.drain()
                # halving-add reduce over C
                width = C
                while width > 1:
                    half = width // 2
                    a_lo = _ap(pa_sb, 0,
                               [[ch_cols * 3, 128], [C * 3, chunk_blocks],
                                [3, half], [1, 3]])
                    a_hi = _ap(pa_sb, half * 3,
                               [[ch_cols * 3, 128], [C * 3, chunk_blocks],
                                [3, half], [1, 3]])
                    vector.tensor_tensor(out=a_lo, in0=a_lo, in1=a_hi, op=AL.add)
                    vector.drain()
                    width = half
                dst_sums = _ap(sums_sb, ch * chunk_blocks * 3,
                               [[B * 3, 128], [3, chunk_blocks], [1, 3]])
                src_sums = _ap(pa_sb, 0,
                               [[ch_cols * 3, 128], [C * 3, chunk_blocks], [1, 3]])
                vector.tensor_copy(out=dst_sums, in_=src_sums)
                vector.drain().then_inc(v_sem, 1)
            # final combine
            vector.tensor_scalar_min(out=t0_sb[:], in0=cnt_sb[:], scalar1=1.0)
            vector.tensor_scalar_max(out=t1_sb[:], in0=cnt_sb[:], scalar1=1.0)
            vector.drain()
            vector.reciprocal(out=t1_sb[:], in_=t1_sb[:])
            vector.drain()
            vector.tensor_tensor(out=t1_sb[:], in0=t1_sb[:], in1=nf_sb[:],
                                 op=AL.mult)
            vector.drain()
            o0 = _ap(o_sb, 0, [[B * 4, 128], [4, B]])
            w0b = _ap(w_sb, 0, [[4, 128], [0, B]])
            vector.tensor_tensor(out=o0, in0=t0_sb[:], in1=nf_sb[:], op=AL.mult)
            vector.drain()
            vector.tensor_tensor(out=o0, in0=o0, in1=w0b, op=AL.mult)
            vector.drain()
            for c in range(3):
                oc = _ap(o_sb, 1 + c, [[B * 4, 128], [4, B]])
                sc = _ap(sums_sb, c, [[B * 3, 128], [3, B]])
                wcb = _ap(w_sb, 1 + c, [[4, 128], [0, B]])
                vector.tensor_tensor(out=oc, in0=sc, in1=t1_sb[:], op=AL.mult)
                vector.drain()
                vector.tensor_tensor(out=oc, in0=oc, in1=wcb, op=AL.mult)
                vector.drain()
            vector.drain().then_inc(v_sem, 1)

        @block.scalar
        def _(scalar):
            for ch in range(n_chunks):
                scalar.wait_ge(a_sem, 2 * ch + 1)
                scalar.activation(
                    out=inv_sb[:], in_=ss_sb[:],
                    func=mybir.ActivationFunctionType.Sqrt,
                    bias=EPS2, scale=1.0,
                ).then_inc(a_sem, 1)

    nc.compile()
    _PROG_CACHE[key] = nc
    return nc


def host_prep(positions, node_feat, w0, w1, edge_src, edge_dst, C):
    pos = np.ascontiguousarray(positions, dtype=np.float32)
    f = np.ascontiguousarray(node_feat, dtype=np.float32).reshape(-1)
    src = np.asarray(edge_src).astype(np.int64)
    dst = np.asarray(edge_dst).astype(np.int64)

    NT = NC * NPC
    counts = np.bincount(dst, minlength=NT).astype(np.int64)

    order = np.argsort(dst, kind="stable")
    dst_s = dst[order]
    src_s = src[order]
    starts = np.zeros(NT + 1, dtype=np.int64)
    np.cumsum(counts, out=starts[1:])
    slot_of_edge = np.arange(len(dst_s)) - starts[dst_s]
    slot_src = np.repeat(np.arange(NT, dtype=np.int64), C).reshape(NT, C)
    slot_src[dst_s, slot_of_edge] = src_s

    ptab = np.zeros((NREC, 64), dtype=np.float32)
    pos_pad = np.zeros((NREC * 4, 3), dtype=np.float32)
    pos_pad[:N_NODES] = pos
    ptab[:, :12] = pos_pad.reshape(NREC, 12)

    in_maps = []
    cols = B * C
    wvec = np.tile(
        np.concatenate([np.asarray(w0, np.float32).reshape(1),
                        np.asarray(w1, np.float32).reshape(3)]).reshape(1, 4),
        (P, 1)).astype(np.float32)
    for k in range(NC):
        lo = k * NPC
        nodes = np.arange(lo, lo + NPC)
        n_local = nodes - lo
        pmap = n_local % P
        bmap = n_local // P

        ssrc = np.zeros((P, B, C), dtype=np.int64)
        ssrc[pmap, bmap] = slot_src[nodes]
        ssrc = ssrc.reshape(P, cols)

        stream = ssrc.T.reshape(-1)                  # i = col*128 + p
        rec_idx = (stream >> 2).astype(np.int16)
        wrapped = rec_idx.reshape(-1, 16).T          # [16, len/16]
        idx_w = np.ascontiguousarray(np.tile(wrapped, (8, 1)), dtype=np.int16)

        low2 = (ssrc & 3)
        mks = np.zeros((P, 4, cols), dtype=np.float32)
        for kk in range(4):
            mks[:, kk, :] = (low2 == kk)

        valid = nodes < N_NODES
        pd = np.zeros((P, B, 3), dtype=np.float32)
        pd[pmap[valid], bmap[valid]] = pos[nodes[valid]]
        cn = np.zeros((P, B), dtype=np.float32)
        cn[pmap, bmap] = counts[nodes].astype(np.float32)
        nf = np.zeros((P, B), dtype=np.float32)
        nf[pmap[valid], bmap[valid]] = f[nodes[valid]]

        in_maps.append({
            "ptab": ptab, "idxs": idx_w, "masks": mks,
            "pdst": pd, "cnts": cn, "nfeat": nf, "wvec": wvec,
        })
    return in_maps


def kernel(positions, node_feat, w0, w1, edge_src, edge_dst):
    dst = np.asarray(edge_dst).astype(np.int64)
    maxdeg = int(np.bincount(dst, minlength=N_NODES).max())
    C = 64
    while C < maxdeg:
        C *= 2
    chunk_blocks = 14 if B % 14 == 0 else 7

    in_maps = host_prep(positions, node_feat, w0, w1, edge_src, edge_dst, C)
    nc = build_program(C, chunk_blocks)
    res = run_bass_kernel_spmd(nc, in_maps, core_ids=list(range(NC)))

    full = np.zeros((NC * NPC, 4), dtype=np.float32)
    n_local = np.arange(NPC)
    for k in range(NC):
        o = res.results[k]["out"]
        full[k * NPC + n_local] = o[n_local % P, n_local // P, :]
    return full[:N_NODES]
